# revision 57
# baseline (speedup 1.0000x reference)
"""ButterFlyNet2D forward on 8 trn2 NeuronCores.

Sharding: core c handles layer-1 parent block (u0,v0) = divmod(c//2, 2) and
m-half ly = c%2 (2 of 16 layer-2 subtrees), full batch. Butterfly weights are
read exactly once across the 8 cores.

Numerics: single-pass bf16 matmuls everywhere (incl. the final 1x1 layer);
activations stored bf16. Measured rel err vs fp32 reference ~7.4e-3 (gate is
2e-2; inputs are deterministic so this is stable).

Activation layout: SBUF tiles [128 partitions = 64*q + n, cols], where (p, q)
are the 2x2 patch offsets of the NEXT conv layer (p picks the tile set, q the
partition half); cols = block-major: ib*npos + b*(s/2)^2 + y2*(s/2) + x2,
chopped into [128, 2048] tiles.

Schedule/perf structure:
- weight DMAs batched 2 blocks/DMA, issued on BOTH HWDGE queues (SP + Act),
  with deep prefetch rings (l5: 40 groups) so the 21MB/core weight stream
  never stalls the tensor engine;
- layers 3-5 + final run depth-first per fill chain (l3 f -> l4 f -> l5 f ->
  final f) so l5 weight-ring consumption starts early;
- butterfly scatter goes psum -> bf16 staging (2 wide relu ops, frees PSUM
  for the next fill fast) -> strided 64-partition copies alternating between
  the Act and DVE engines ("bounce" mode);
- final 1x1 layer computed as 4 wide block-diagonal matmuls per fill
  (M=128 = 16 slots x 8 outs); the full [128,2048] product is DMA'd out and
  the host extracts the diagonal blocks (+relu) in decode_outputs.
"""

import numpy as np
import ml_dtypes

# ---------------------------------------------------------------- constants
B, C, H, W, L, T = 32, 1, 64, 64, 6, 4
NCH = 64
KO = 256
N_CORES = 8
FILL_W = 2048
TILE_W = 2048

LAYER_S = {1: 32, 2: 16, 3: 8, 4: 4, 5: 2}
LAYER_NPOS = {l: 32 * (LAYER_S[l] // 2) ** 2 for l in LAYER_S}
LAYER_M = {1: 128, 2: 256, 3: 256, 4: 256, 5: 256}
LAYER_NB = {1: 1, 2: 2, 3: 8, 4: 32, 5: 128}
NTILES = {1: 4, 2: 2, 3: 2, 4: 2, 5: 2}  # act tiles per (hl, p)

BF16 = ml_dtypes.bfloat16
AL_ENGINE = "vector"   # "gpsimd" or "vector"

# weight-fetch grouping (blocks per DMA) and ring depth (in group units)
QGRP = {1: 1, 2: 1, 3: 2, 4: 2, 5: 2}
WBUFS = {1: 1, 2: 2, 3: 2, 4: 6, 5: 40}
RELU_SPLIT = 1   # pieces per big relu copy (l5 fact / final fo)
SCATTER_MODE = "bounce"  # "bounce" | "full" | "fused2" (timing probe only)
BOUNCE_SPLIT = 4   # stage-1 psum->staging ops per fill (psum release grain)
STG_BUFS = 2       # staging ring depth
ACT_SHARE = 2      # of every 4 scatter copies, how many go to Act engine


def core_geom(c):
    P, ly = divmod(c, 2)
    u0, v0 = divmod(P, 2)
    blocks = {1: [(u0, v0)]}
    for l in range(2, 6):
        ms = [ly] if l - 1 == 1 else [0, 1]
        nxt = []
        for (u, v) in blocks[l - 1]:
            for a in ms:
                for bb in (0, 1):
                    nxt.append((2 * u + a, 2 * v + bb))
        blocks[l] = nxt
    return u0, v0, ly, blocks


def m_list(l, ly):
    return [ly] if l == 1 else [0, 1]


def child_index(l, ib, m, klx):
    return klx if l == 1 else ib * 4 + m * 2 + klx


def prod_units(l, ly):
    return [(ib, m) for ib in range(LAYER_NB[l]) for m in m_list(l, ly)]


def split_hilo(a):
    hi = a.astype(BF16)
    lo = (a.astype(np.float32) - hi.astype(np.float32)).astype(BF16)
    return hi, lo


# ---------------------------------------------------------------- host packing
def pack_weights_layer(Wl, l, blocks_l, ly):
    """-> [nb, 128, 2*M] bf16; partition = 64*q + n; free = p*M + k."""
    M = LAYER_M[l]
    out = np.zeros((len(blocks_l), 128, 2 * M), dtype=BF16)
    for i, (u, v) in enumerate(blocks_l):
        wb = np.asarray(Wl[0, u, v], dtype=np.float32)      # [256, 64, 2, 2]
        if l == 1:
            wb = wb[ly * 128:(ly + 1) * 128]
        wt = wb.transpose(3, 1, 2, 0).reshape(128, 2, M)     # (q,n), p, k
        out[i] = wt.reshape(128, 2 * M).astype(BF16)
    return out


def pack_first(W0, u0, v0):
    """-> lhsT [8, 128] bf16, block-diag over 2 chunks."""
    koff = (u0 * 2 + v0) * 64
    w0e = np.asarray(W0[0, koff:koff + 64, 0], dtype=np.float32)  # [64, 2, 2]
    wt = w0e.reshape(64, 4).T                                     # [4(hw), 64]
    out = np.zeros((8, 128), dtype=BF16)
    out[0:4, 0:64] = wt.astype(BF16)
    out[4:8, 64:128] = wt.astype(BF16)
    return out


def pack_patches(x):
    """-> [8, 16384] bf16; col = pair*512 + yloc*32 + X;
    rows 0:4 = chunk 2i (Yhalf 0), 4:8 = chunk 2i+1."""
    xs = np.asarray(x[:, 0], dtype=np.float32)
    p = xs.reshape(B, 32, 2, 32, 2).transpose(2, 4, 0, 1, 3).reshape(4, B, 32, 32)
    out = np.zeros((8, 16384), dtype=BF16)
    for i in range(32):
        for cp in range(2):
            sl = np.s_[:, i, cp * 16:(cp + 1) * 16, :]
            out[cp * 4:(cp + 1) * 4, i * 512:(i + 1) * 512] = \
                p[sl].reshape(4, 512).astype(BF16)
    return out


def pack_wf(Wf, blocks5):
    """-> [128, 2048] bf16 block-diag pairs; slot idx: cols [8*idx, 8*idx+8),
    rows 0:64 = Wf(klx=0 block).T at cols 0:4, rows 64:128 = klx=1 at 4:8."""
    out = np.zeros((128, 2048), dtype=BF16)
    for idx in range(256):
        ib, m = idx // 2, idx % 2
        u, v = blocks5[ib]
        for klx in range(2):
            wft = np.asarray(Wf[0, 2 * u + m, 2 * v + klx], np.float32)  # [4,64]
            out[klx * 64:(klx + 1) * 64,
                idx * 8 + klx * 4:idx * 8 + klx * 4 + 4] = wft.T
    return out


# ------------------------------------------------------- scatter descriptors
# copy = (src_pbase, src_off, src_ap, dst_pbase, dst_off_rel, dst_ap)
# region = dict(p2, g, dst_start(local col in tile), width, copies)
def first_fill_descs(f):
    regions = []
    for p2 in range(2):
        base = 1024 * f
        copies = []
        for cp in range(2):
            for q2 in range(2):
                copies.append((cp * 64, p2 * 32 + q2, [[512, 4], [64, 8], [2, 16]],
                               q2 * 64, cp * 128, [[256, 4], [16, 8], [1, 16]]))
        regions.append(dict(p2=p2, g=base // TILE_W, dst_start=base % TILE_W,
                            width=1024, copies=copies))
    return regions


def layer_fill_descs(l, f, ly):
    s2 = LAYER_S[l] // 2
    npos_next = (32 * s2 * s2) // 4
    units = prod_units(l, ly)
    regions = []
    for p2 in range(2):
        reg_map = {}

        def add(g, dst_global, copy):
            reg = reg_map.setdefault(g, dict(p2=p2, g=g, copies=[], _glob=[]))
            reg["copies"].append(copy)
            reg["_glob"].append(dst_global)

        for klx in range(2):
            for q2 in range(2):
                if l == 1:
                    ib, m = units[0]
                    ibc = child_index(l, ib, m, klx)
                    dg = ibc * npos_next + 8 * f * 64
                    add(dg // TILE_W, dg,
                        (klx * 64, p2 * 16 + q2, [[256, 8], [32, 8], [2, 8]],
                         q2 * 64, dg, [[64, 8], [8, 8], [1, 8]]))
                elif l == 2:
                    ib, m = units[f]
                    ibc = child_index(l, ib, m, klx)
                    dg = ibc * npos_next
                    add(dg // TILE_W, dg,
                        (klx * 64, p2 * 8 + q2, [[64, 32], [16, 4], [2, 4]],
                         q2 * 64, dg, [[16, 32], [4, 4], [1, 4]]))
                elif l == 3:
                    ib0, m0 = units[4 * f]
                    ibc0 = child_index(l, ib0, m0, klx)
                    for y2 in range(2):
                        dg = ibc0 * npos_next + y2 * 2
                        add(dg // TILE_W, dg,
                            (klx * 64, (2 * y2 + p2) * 4 + q2,
                             [[512, 4], [16, 32], [2, 2]],
                             q2 * 64, dg, [[256, 4], [4, 32], [1, 2]]))
                elif l == 4:
                    ib0, m0 = units[16 * f]
                    ibc0 = child_index(l, ib0, m0, klx)
                    dg = ibc0 * npos_next
                    add(dg // TILE_W, dg,
                        (klx * 64, p2 * 2 + q2, [[128, 16], [4, 32]],
                         q2 * 64, dg, [[64, 16], [1, 32]]))
                else:
                    raise AssertionError(l)
        for reg in reg_map.values():
            base = min(reg["_glob"])
            ext = 0
            fixed = []
            for (spb, soff, sap, dpb, dg, dap), g0 in zip(reg["copies"],
                                                          reg["_glob"]):
                rel = g0 - base
                fixed.append((spb, soff, sap, dpb, rel, dap))
                ext = max(ext, rel + sum(st * (ct - 1) for st, ct in dap) + 1)
            assert (base % TILE_W) + ext <= TILE_W, (l, f, base, ext)
            regions.append(dict(p2=reg["p2"], g=reg["g"],
                                dst_start=base % TILE_W, width=ext,
                                copies=fixed))
        del reg_map
    return regions


def layer_slots(l, ly):
    npos = LAYER_NPOS[l]
    nch = max(1, npos // 512)
    return [(ib, m, chk) for (ib, m) in prod_units(l, ly) for chk in range(nch)]


# ------------------------------------------------------------------ mirror
def _ap_cols(off, ap):
    idx = np.zeros((1,), np.int64) + off
    for stride, count in ap:
        idx = (idx[:, None] + (np.arange(count) * stride)[None, :]).reshape(-1)
    return idx


def mirror_core(inputs, c):
    """Pure-numpy mirror of the device plan for core c -> fout [2,128,2048]."""
    u0, v0, ly, blocks = core_geom(c)
    w0 = pack_first(inputs["W0"], u0, v0).astype(np.float32)
    pat = pack_patches(inputs["input_data"]).astype(np.float32)
    wl = {l: pack_weights_layer(inputs[f"W{l}"], l, blocks[l], ly)
          for l in range(1, 6)}
    wf = pack_wf(inputs["Wf"], blocks[5]).astype(np.float32)

    act = {l: [[np.zeros((128, TILE_W), np.float32) for _ in range(NTILES[l])]
               for _ in range(2)] for l in range(1, 6)}
    fact = [None] * 4

    def apply_regions(psum, regions, l_next):
        for reg in regions:
            for (spb, soff, sap, dpb, doff, dap) in reg["copies"]:
                sc = _ap_cols(soff, sap)
                dc = _ap_cols(reg["dst_start"] + doff, dap)
                vals = np.maximum(psum[spb:spb + 64][:, sc], 0.0)
                vals = vals.astype(BF16).astype(np.float32)
                act[l_next][reg["p2"]][reg["g"]][dpb:dpb + 64][:, dc] = vals

    for f in range(8):
        psum = np.zeros((128, FILL_W), np.float32)
        for s in range(4):
            t = 4 * f + s
            psum[:, s * 512:(s + 1) * 512] = w0.T @ pat[:, t * 512:(t + 1) * 512]
        apply_regions(psum, first_fill_descs(f), 1)

    for l in range(1, 6):
        M = LAYER_M[l]
        npos = LAYER_NPOS[l]
        slots = layer_slots(l, ly)
        w_slot = min(npos, 512)
        spf = FILL_W // w_slot
        nfill = len(slots) // spf
        for f in range(nfill):
            psum = np.zeros((128, FILL_W), np.float32)
            for si in range(spf):
                ib, m, chk = slots[f * spf + si]
                colg = ib * npos + chk * 512
                g, loc = colg // TILE_W, colg % TILE_W
                wb = wl[l][ib].astype(np.float32)
                mh = m * 128 if M == 256 else 0
                out = np.zeros((128, w_slot), np.float32)
                for p in range(2):
                    Wh = wb[:, p * M + mh:p * M + mh + 128]
                    Ah = act[l][p][g][:, loc:loc + w_slot]
                    out += Wh.T @ Ah
                psum[:, si * w_slot:(si + 1) * w_slot] = out
            if l == 5:
                fact[f] = np.maximum(psum, 0.0).astype(BF16).astype(np.float32)
            else:
                apply_regions(psum, layer_fill_descs(l, f, ly), l + 1)

    fout = np.zeros((4, 128, FILL_W), np.float32)
    for fi in range(4):
        for j in range(4):
            g = fi * 4 + j
            prod = wf[:, g * 128:(g + 1) * 128].T @ \
                fact[fi][:, j * 512:(j + 1) * 512]
            fout[fi][:, j * 512:(j + 1) * 512] = \
                np.maximum(prod, 0.0).astype(BF16).astype(np.float32)
    return fout


def decode_outputs(fouts):
    """fouts[c] = [4, 128, 2048]: block-diag final-layer product; slot
    s = 16*j + t of fill fi lives at rows 8t+(klx*4+k), cols 512j+32t+b."""
    out = np.zeros((B, C, 2, 64, 64), np.float32)
    for c, fo in fouts.items():
        fo = np.asarray(fo, np.float32)
        _, _, _, blocks = core_geom(c)
        blocks5 = blocks[5]
        for fi in range(4):
            for s in range(64):
                j, t = divmod(s, 16)
                idx = 64 * fi + s
                ib, m = idx // 2, idx % 2
                u, v = blocks5[ib]
                for klx in range(2):
                    U, V = 2 * u + m, 2 * v + klx
                    r0 = 8 * t + klx * 4
                    c0 = 512 * j + 32 * t
                    yf = np.maximum(fo[fi, r0:r0 + 4, c0:c0 + 32], 0.0)
                    out[:, 0, 0, U, V] = yf[0] - yf[2]
                    out[:, 0, 1, U, V] = yf[1] - yf[3]
    return out


def mirror_forward(inputs, cores=range(N_CORES)):
    return decode_outputs({c: mirror_core(inputs, c) for c in cores})


# ------------------------------------------------------------- numpy fallback
def _numpy_reference(inputs):
    x = np.asarray(inputs["input_data"], np.float32)
    b, c_, h, w = x.shape
    xs = np.zeros((b, c_, 4, h, w), np.float32)
    xs[:, :, 0] = x
    p = xs.reshape(b, c_, 4, 32, 2, 32, 2)
    W0 = np.asarray(inputs["W0"], np.float32)
    b0 = np.asarray(inputs["b0"], np.float32)
    y = np.einsum('bcnYhXw,cknhw->bckYX', p, W0) + b0[None, :, :, None, None]
    state = np.maximum(y, 0).reshape(b, c_, 2, 2, NCH, 32, 32)
    for l in range(1, 6):
        Wl = np.asarray(inputs[f"W{l}"], np.float32)
        bl = np.asarray(inputs[f"b{l}"], np.float32)
        G = Wl.shape[1]
        s = state.shape[-1]
        s2 = s // 2
        p = state.reshape(b, c_, G, G, NCH, s2, 2, s2, 2)
        y = np.einsum('bcuvnYpXq,cuvknpq->bcuvkYX', p, Wl) + \
            bl[None, :, :, :, :, None, None]
        y = np.maximum(y, 0).reshape(b, c_, G, G, 2, 2, NCH, s2, s2)
        y = y.transpose(0, 1, 2, 4, 3, 5, 6, 7, 8)
        state = y.reshape(b, c_, 2 * G, 2 * G, NCH, s2, s2)
    st = state.reshape(b, c_, 64, 64, NCH)
    Wf = np.asarray(inputs["Wf"], np.float32)
    bf = np.asarray(inputs["bf"], np.float32)
    yf = np.maximum(np.einsum('bcuvn,cuvkn->bcuvk', st, Wf) + bf[None], 0)
    real = yf[..., 0] - yf[..., 2]
    imag = yf[..., 1] - yf[..., 3]
    return np.stack([real, imag], axis=2)


# ------------------------------------------------------------- bass program
_NC_CACHE = {}


def build_nc(stop_after=None, loop=False):
    import concourse.bass as bass
    import concourse.mybir as mybir
    import concourse.tile as tile
    from concourse import bacc
    import contextlib

    F32 = mybir.dt.float32
    BF = mybir.dt.bfloat16
    Relu = mybir.ActivationFunctionType.Relu

    nc = bacc.Bacc(None, target_bir_lowering=False, debug=True)

    d_pat = nc.dram_tensor("patches", [8, 16384], BF, kind="ExternalInput")
    d_w0 = nc.dram_tensor("w0", [8, 128], BF, kind="ExternalInput")
    d_wl = {l: nc.dram_tensor(f"w{l}", [LAYER_NB[l], 128, 2 * LAYER_M[l]], BF,
                              kind="ExternalInput") for l in range(1, 6)}
    d_wf = nc.dram_tensor("wf", [128, 2048], BF, kind="ExternalInput")
    d_out = nc.dram_tensor("fout", [4, 128, FILL_W], BF, kind="ExternalOutput")
    if loop:
        d_bound = nc.dram_tensor("bound", [1, 1], mybir.dt.int32,
                                 kind="ExternalInput")

    with tile.TileContext(nc) as tc:
        with contextlib.ExitStack() as ctx:
            ps = ctx.enter_context(tc.tile_pool(name="ps", bufs=2, space="PSUM"))
            sb = ctx.enter_context(tc.tile_pool(name="sb", bufs=1))
            wpool = ctx.enter_context(tc.tile_pool(name="wp", bufs=1))

            loop_cm = contextlib.nullcontext()
            if loop:
                bt = sb.tile([1, 1], mybir.dt.int32, tag="bt", bufs=1)
                nc.sync.dma_start(out=bt[:], in_=d_bound[:])
                nval = nc.values_load(bt[0:1, 0:1], min_val=0, max_val=1000000,
                                      skip_runtime_bounds_check=True)
                loop_cm = tc.For_i(0, nval, 1)
            ctx.enter_context(loop_cm)

            w0_sb = sb.tile([8, 128], BF, tag="w0", bufs=1)
            nc.sync.dma_start(out=w0_sb[:], in_=d_w0[:])
            pat_sb = []
            for i in range(4):
                t = sb.tile([8, 4096], BF, tag="pat", bufs=2, name=f"pat{i}")
                nc.sync.dma_start(out=t[:], in_=d_pat[:, i * 4096:(i + 1) * 4096])
                pat_sb.append(t)
            wf_sb = sb.tile([128, 2048], BF, tag="wf", bufs=1)
            nc.sync.dma_start(out=wf_sb[:], in_=d_wf[:])

            act = {l: [[None] * NTILES[l] for _ in range(2)]
                   for l in range(1, 6)}

            def act_tile(l, p, g):
                if act[l][p][g] is None:
                    act[l][p][g] = sb.tile(
                        [128, TILE_W], BF, tag="act", bufs=12,
                        name=f"act{l}_{p}{g}")
                return act[l][p][g]

            scat_flip = [0]

            def relu_copy(dst, src):
                if scat_flip[0] % 4 < ACT_SHARE:
                    nc.scalar.activation(dst, src, Relu)
                else:
                    nc.vector.tensor_scalar_max(dst, src, 0.0)
                scat_flip[0] += 1

            def emit_scatter(psum, regions, l_next):
                if SCATTER_MODE == "fused2":
                    for reg in regions:
                        p2, g, st_loc = reg["p2"], reg["g"], reg["dst_start"]
                        w = min(reg["width"], FILL_W)
                        ah = act_tile(l_next, p2, g)
                        relu_copy(ah[:, st_loc:st_loc + w], psum[:, 0:w])
                    return
                deint = SCATTER_MODE == "bounce2"
                if SCATTER_MODE == "bounce":
                    # relu psum -> bf16 staging in wide ops (frees PSUM
                    # fast); the strided scatter then reads staging
                    stg = sb.tile([128, FILL_W], BF, tag="stg",
                                  bufs=STG_BUFS, name="stg")
                    seg = FILL_W // BOUNCE_SPLIT
                    for h in range(BOUNCE_SPLIT):
                        relu_copy(stg[:, h * seg:(h + 1) * seg],
                                  psum[:, h * seg:(h + 1) * seg])
                    s_tile, s_w = stg, FILL_W
                elif deint:
                    # like bounce, but staging de-interleaves x-parity:
                    # stg col = (c >> 1) + (c & 1)*1024, so the scatter's
                    # inner dims become contiguous (DVE 2x eligible)
                    stg = sb.tile([128, FILL_W], BF, tag="stg", bufs=2,
                                  name="stg")
                    for h in range(2):
                        src = bass.AP(tensor=psum[:].tensor,
                                      offset=psum[:].offset + h * 1024,
                                      ap=[[FILL_W, 128], [2, 512], [1, 2]])
                        dst = bass.AP(tensor=stg[:].tensor,
                                      offset=stg[:].offset + h * 512,
                                      ap=[[FILL_W, 128], [1, 512], [1024, 2]])
                        relu_copy(dst, src)
                    s_tile, s_w = stg, FILL_W
                else:
                    s_tile, s_w = psum, FILL_W
                for reg in regions:
                    p2, g, st_loc = reg["p2"], reg["g"], reg["dst_start"]
                    ah = act_tile(l_next, p2, g)
                    for (spb, soff, sap, dpb, doff, dap) in reg["copies"]:
                        if deint:
                            soff2 = (soff >> 1) + (soff & 1) * 1024
                            sap2 = [[st // 2, ct] for st, ct in sap]
                        else:
                            soff2, sap2 = soff, [list(x) for x in sap]
                        src = bass.AP(
                            tensor=s_tile[:].tensor,
                            offset=s_tile[:].offset + spb * s_w + soff2,
                            ap=[[s_w, 64]] + sap2)
                        dst_h = bass.AP(
                            tensor=ah[:].tensor,
                            offset=ah[:].offset + dpb * TILE_W + st_loc + doff,
                            ap=[[TILE_W, 64]] + [list(x) for x in dap])
                        if scat_flip[0] % 2 == 0:
                            nc.scalar.activation(dst_h, src, Relu)
                        else:
                            nc.vector.tensor_scalar_max(dst_h, src, 0.0)
                        scat_flip[0] += 1

            # first layer
            for f in range(8):
                psum = ps.tile([128, FILL_W], F32, tag="ps", bufs=2, name="psF")
                for s in range(4):
                    t = 4 * f + s
                    rhs = pat_sb[t // 8][:, (t % 8) * 512:(t % 8) * 512 + 512]
                    nc.tensor.matmul(psum[:, s * 512:(s + 1) * 512],
                                     w0_sb[:], rhs, start=True, stop=True)
                emit_scatter(psum, first_fill_descs(f), 1)

            # recursion layers (program identical across cores; ly only
            # affects the data packed on the host)
            fact_tiles = []
            # weight tiles are fetched QGRP blocks per DMA (contiguous in
            # DRAM) to cut DMA queue overhead; bufs are in group units
            wbufs = WBUFS
            w_sbs = {l: {} for l in range(1, 6)}

            def emit_fill(l, f):
                M = LAYER_M[l]
                npos = LAYER_NPOS[l]
                slots = layer_slots(l, 0)
                w_slot = min(npos, 512)
                spf = FILL_W // w_slot
                w_sb = w_sbs[l]
                psum = ps.tile([128, FILL_W], F32, tag="ps", bufs=2,
                               name=f"psl{l}")
                fill_slots = slots[f * spf:(f + 1) * spf]
                Q = QGRP[l]
                for (ib, m, chk) in fill_slots:
                    qi = ib // Q
                    if qi not in w_sb:
                        wt = wpool.tile([128, Q * 2 * M], BF, tag=f"w{l}",
                                        bufs=wbufs[l], name=f"w{l}_q{qi}")
                        src = d_wl[l][qi * Q:(qi + 1) * Q]
                        if Q > 1:
                            src = src.rearrange("i p c -> p i c")
                        dma_eng = nc.scalar if qi % 2 else nc.sync
                        dma_eng.dma_start(out=wt[:], in_=src)
                        w_sb[qi] = wt
                for si, (ib, m, chk) in enumerate(fill_slots):
                    wt = w_sb[ib // Q]
                    wbase = (ib % Q) * 2 * M
                    colg = ib * npos + chk * 512
                    g, loc = colg // TILE_W, colg % TILE_W
                    mh = m * 128 if M == 256 else 0
                    pslice = psum[:, si * w_slot:(si + 1) * w_slot]
                    for p in range(2):
                        lhsT = wt[:, wbase + p * M + mh:
                                  wbase + p * M + mh + 128]
                        rhs = act_tile(l, p, g)[:, loc:loc + w_slot]
                        nc.tensor.matmul(pslice, lhsT, rhs,
                                         start=(p == 0), stop=(p == 1))
                if l == 5:
                    ft = sb.tile([128, FILL_W], BF, tag="fact", bufs=4,
                                 name=f"fact{f}")
                    wseg = FILL_W // RELU_SPLIT
                    for rs in range(RELU_SPLIT):
                        relu_copy(ft[:, rs * wseg:(rs + 1) * wseg],
                                  psum[:, rs * wseg:(rs + 1) * wseg])
                    fact_tiles.append(ft)
                else:
                    emit_scatter(psum, layer_fill_descs(l, f, 0), l + 1)

            def emit_final(fi):
                # final layer: 4 wide matmuls per fill (M=128 = 16 slots x
                # 8 outs, moving 512 = 16 slots x 32 batch); only the
                # diagonal [8t:8t+8, 32t:32t+32] blocks are wanted -- a
                # strided DMA gathers them into d_out (relu'd in SBUF;
                # host applies relu again, idempotent).
                psF = ps.tile([128, FILL_W], F32, tag="ps", bufs=2,
                              name="psfin")
                for j in range(4):
                    g = fi * 4 + j
                    lhsT = wf_sb[:, g * 128:(g + 1) * 128]
                    rhs = fact_tiles[fi][:, j * 512:(j + 1) * 512]
                    nc.tensor.matmul(psF[:, j * 512:(j + 1) * 512],
                                     lhsT, rhs, start=True, stop=True)
                fo = sb.tile([128, FILL_W], BF, tag="fo", bufs=2,
                             name=f"fout{fi}")
                wseg = FILL_W // RELU_SPLIT
                for rs in range(RELU_SPLIT):
                    relu_copy(fo[:, rs * wseg:(rs + 1) * wseg],
                              psF[:, rs * wseg:(rs + 1) * wseg])
                nc.sync.dma_start(out=d_out[fi], in_=fo[:])

            # layers 1-2 layer-major; layers 3-5 + final depth-first per
            # fill chain so l5 weight-ring consumption starts early and
            # the weight DMA stream never stalls
            lvl = {None: 99, "first": 0, "l1": 1, "l2": 2, "l3": 3,
                   "l4": 4, "l5": 5}[stop_after]
            for l in (1, 2):
                if lvl >= l:
                    for f in range(4):
                        emit_fill(l, f)
            for f in range(4):
                for l in (3, 4, 5):
                    if lvl >= l:
                        emit_fill(l, f)
                if stop_after == "l5":
                    nc.sync.dma_start(out=d_out[f], in_=fact_tiles[f][:])
                elif stop_after is None:
                    emit_final(f)
    nc.finalize()
    return nc


# ------------------------------------------------------------------ kernel()
def _pack_in_maps(inputs):
    pat = pack_patches(inputs["input_data"])
    in_maps = []
    for c in range(N_CORES):
        u0, v0, ly, blocks = core_geom(c)
        m = {"patches": pat,
             "w0": pack_first(inputs["W0"], u0, v0),
             "wf": pack_wf(inputs["Wf"], blocks[5])}
        for l in range(1, 6):
            m[f"w{l}"] = pack_weights_layer(inputs[f"W{l}"], l, blocks[l], ly)
        in_maps.append(m)
    return in_maps


def kernel(**inputs):
    exp = {"input_data": (B, C, H, W), "W0": (C, KO, 4, 2, 2), "b0": (C, KO),
           "Wf": (C, 64, 64, 4, NCH), "bf": (C, 64, 64, 4)}
    for l in range(1, 6):
        G = 2 ** l
        exp[f"W{l}"] = (C, G, G, KO, NCH, 2, 2)
        exp[f"b{l}"] = (C, G, G, KO)
    ok = all(tuple(np.shape(inputs.get(k, ()))) == v for k, v in exp.items())
    biases_zero = all(not np.any(np.asarray(inputs[k]))
                      for k in inputs if k.startswith("b"))
    if not ok or not biases_zero:
        return _numpy_reference(inputs)

    from concourse.bass_utils import run_bass_kernel_spmd

    if "nc" not in _NC_CACHE:
        _NC_CACHE["nc"] = build_nc()
    res = run_bass_kernel_spmd(_NC_CACHE["nc"], _pack_in_maps(inputs),
                               core_ids=list(range(N_CORES)))
    return decode_outputs({c: res.results[c]["fout"] for c in range(N_CORES)})



# revision 59
# speedup vs baseline: 1.0415x; 1.0415x over previous
"""ButterFlyNet2D forward on 8 trn2 NeuronCores.

Sharding: core c handles layer-1 parent block (u0,v0) = divmod(c//2, 2) and
m-half ly = c%2 (2 of 16 layer-2 subtrees), full batch. Butterfly weights are
read exactly once across the 8 cores.

Numerics: single-pass bf16 matmuls everywhere (incl. the final 1x1 layer);
activations stored bf16. Measured rel err vs fp32 reference ~7.4e-3 (gate is
2e-2; inputs are deterministic so this is stable).

Activation layout: SBUF tiles [128 partitions = 64*q + n, cols], where (p, q)
are the 2x2 patch offsets of the NEXT conv layer (p picks the tile set, q the
partition half); cols = block-major: ib*npos + b*(s/2)^2 + y2*(s/2) + x2,
chopped into [128, 2048] tiles.

Schedule/perf structure:
- weight DMAs batched 2 blocks/DMA, issued on BOTH HWDGE queues (SP + Act),
  with deep prefetch rings (l5: 40 groups) so the 21MB/core weight stream
  never stalls the tensor engine;
- layers 3-5 + final run depth-first per fill chain (l3 f -> l4 f -> l5 f ->
  final f) so l5 weight-ring consumption starts early;
- butterfly scatter goes psum -> bf16 staging (2 wide relu ops, frees PSUM
  for the next fill fast) -> strided 64-partition copies alternating between
  the Act and DVE engines ("bounce" mode);
- final 1x1 layer computed as 4 wide block-diagonal matmuls per fill
  (M=128 = 16 slots x 8 outs); the full [128,2048] product is DMA'd out and
  the host extracts the diagonal blocks (+relu) in decode_outputs.
"""

import numpy as np
import ml_dtypes

# ---------------------------------------------------------------- constants
B, C, H, W, L, T = 32, 1, 64, 64, 6, 4
NCH = 64
KO = 256
N_CORES = 8
FILL_W = 2048
TILE_W = 2048

LAYER_S = {1: 32, 2: 16, 3: 8, 4: 4, 5: 2}
LAYER_NPOS = {l: 32 * (LAYER_S[l] // 2) ** 2 for l in LAYER_S}
LAYER_M = {1: 128, 2: 256, 3: 256, 4: 256, 5: 256}
LAYER_NB = {1: 1, 2: 2, 3: 8, 4: 32, 5: 128}
NTILES = {1: 4, 2: 2, 3: 2, 4: 2, 5: 2}  # act tiles per (hl, p)

BF16 = ml_dtypes.bfloat16
AL_ENGINE = "vector"   # "gpsimd" or "vector"

# weight-fetch grouping (blocks per DMA) and ring depth (in group units)
QGRP = {1: 1, 2: 1, 3: 2, 4: 2, 5: 2}
WBUFS = {1: 1, 2: 2, 3: 2, 4: 6, 5: 40}
RELU_SPLIT = 1   # pieces per big relu copy (l5 fact / final fo)
SCATTER_MODE = "bounce"  # "bounce" | "full" | "fused2" (timing probe only)
BOUNCE_SPLIT = 4   # stage-1 psum->staging ops per fill (psum release grain)
STG_BUFS = 2       # staging ring depth
ACT_SHARE = 2      # of every 4 scatter copies, how many go to Act engine
POOL_SHARE = 0     # of every 4 stage-2 copies, how many go to Pool/gpsimd
                   # (legal only in bounce mode: src is SBUF, not PSUM)


def core_geom(c):
    P, ly = divmod(c, 2)
    u0, v0 = divmod(P, 2)
    blocks = {1: [(u0, v0)]}
    for l in range(2, 6):
        ms = [ly] if l - 1 == 1 else [0, 1]
        nxt = []
        for (u, v) in blocks[l - 1]:
            for a in ms:
                for bb in (0, 1):
                    nxt.append((2 * u + a, 2 * v + bb))
        blocks[l] = nxt
    return u0, v0, ly, blocks


def m_list(l, ly):
    return [ly] if l == 1 else [0, 1]


def child_index(l, ib, m, klx):
    return klx if l == 1 else ib * 4 + m * 2 + klx


def prod_units(l, ly):
    return [(ib, m) for ib in range(LAYER_NB[l]) for m in m_list(l, ly)]


def split_hilo(a):
    hi = a.astype(BF16)
    lo = (a.astype(np.float32) - hi.astype(np.float32)).astype(BF16)
    return hi, lo


# ---------------------------------------------------------------- host packing
def pack_weights_layer(Wl, l, blocks_l, ly):
    """-> [nb, 128, 2*M] bf16; partition = 64*q + n; free = p*M + k."""
    M = LAYER_M[l]
    out = np.zeros((len(blocks_l), 128, 2 * M), dtype=BF16)
    for i, (u, v) in enumerate(blocks_l):
        wb = np.asarray(Wl[0, u, v], dtype=np.float32)      # [256, 64, 2, 2]
        if l == 1:
            wb = wb[ly * 128:(ly + 1) * 128]
        wt = wb.transpose(3, 1, 2, 0).reshape(128, 2, M)     # (q,n), p, k
        out[i] = wt.reshape(128, 2 * M).astype(BF16)
    return out


def pack_first(W0, u0, v0):
    """-> lhsT [8, 128] bf16, block-diag over 2 chunks."""
    koff = (u0 * 2 + v0) * 64
    w0e = np.asarray(W0[0, koff:koff + 64, 0], dtype=np.float32)  # [64, 2, 2]
    wt = w0e.reshape(64, 4).T                                     # [4(hw), 64]
    out = np.zeros((8, 128), dtype=BF16)
    out[0:4, 0:64] = wt.astype(BF16)
    out[4:8, 64:128] = wt.astype(BF16)
    return out


def pack_patches(x):
    """-> [8, 16384] bf16; col = pair*512 + yloc*32 + X;
    rows 0:4 = chunk 2i (Yhalf 0), 4:8 = chunk 2i+1."""
    xs = np.asarray(x[:, 0], dtype=np.float32)
    p = xs.reshape(B, 32, 2, 32, 2).transpose(2, 4, 0, 1, 3).reshape(4, B, 32, 32)
    out = np.zeros((8, 16384), dtype=BF16)
    for i in range(32):
        for cp in range(2):
            sl = np.s_[:, i, cp * 16:(cp + 1) * 16, :]
            out[cp * 4:(cp + 1) * 4, i * 512:(i + 1) * 512] = \
                p[sl].reshape(4, 512).astype(BF16)
    return out


def pack_wf(Wf, blocks5):
    """-> [128, 2048] bf16 block-diag pairs; slot idx: cols [8*idx, 8*idx+8),
    rows 0:64 = Wf(klx=0 block).T at cols 0:4, rows 64:128 = klx=1 at 4:8."""
    out = np.zeros((128, 2048), dtype=BF16)
    for idx in range(256):
        ib, m = idx // 2, idx % 2
        u, v = blocks5[ib]
        for klx in range(2):
            wft = np.asarray(Wf[0, 2 * u + m, 2 * v + klx], np.float32)  # [4,64]
            out[klx * 64:(klx + 1) * 64,
                idx * 8 + klx * 4:idx * 8 + klx * 4 + 4] = wft.T
    return out


# ------------------------------------------------------- scatter descriptors
# copy = (src_pbase, src_off, src_ap, dst_pbase, dst_off_rel, dst_ap)
# region = dict(p2, g, dst_start(local col in tile), width, copies)
def first_fill_descs(f):
    regions = []
    for p2 in range(2):
        base = 1024 * f
        copies = []
        for cp in range(2):
            for q2 in range(2):
                copies.append((cp * 64, p2 * 32 + q2, [[512, 4], [64, 8], [2, 16]],
                               q2 * 64, cp * 128, [[256, 4], [16, 8], [1, 16]]))
        regions.append(dict(p2=p2, g=base // TILE_W, dst_start=base % TILE_W,
                            width=1024, copies=copies))
    return regions


def layer_fill_descs(l, f, ly):
    s2 = LAYER_S[l] // 2
    npos_next = (32 * s2 * s2) // 4
    units = prod_units(l, ly)
    regions = []
    for p2 in range(2):
        reg_map = {}

        def add(g, dst_global, copy):
            reg = reg_map.setdefault(g, dict(p2=p2, g=g, copies=[], _glob=[]))
            reg["copies"].append(copy)
            reg["_glob"].append(dst_global)

        for klx in range(2):
            for q2 in range(2):
                if l == 1:
                    ib, m = units[0]
                    ibc = child_index(l, ib, m, klx)
                    dg = ibc * npos_next + 8 * f * 64
                    add(dg // TILE_W, dg,
                        (klx * 64, p2 * 16 + q2, [[256, 8], [32, 8], [2, 8]],
                         q2 * 64, dg, [[64, 8], [8, 8], [1, 8]]))
                elif l == 2:
                    ib, m = units[f]
                    ibc = child_index(l, ib, m, klx)
                    dg = ibc * npos_next
                    add(dg // TILE_W, dg,
                        (klx * 64, p2 * 8 + q2, [[64, 32], [16, 4], [2, 4]],
                         q2 * 64, dg, [[16, 32], [4, 4], [1, 4]]))
                elif l == 3:
                    ib0, m0 = units[4 * f]
                    ibc0 = child_index(l, ib0, m0, klx)
                    for y2 in range(2):
                        dg = ibc0 * npos_next + y2 * 2
                        add(dg // TILE_W, dg,
                            (klx * 64, (2 * y2 + p2) * 4 + q2,
                             [[512, 4], [16, 32], [2, 2]],
                             q2 * 64, dg, [[256, 4], [4, 32], [1, 2]]))
                elif l == 4:
                    ib0, m0 = units[16 * f]
                    ibc0 = child_index(l, ib0, m0, klx)
                    dg = ibc0 * npos_next
                    add(dg // TILE_W, dg,
                        (klx * 64, p2 * 2 + q2, [[128, 16], [4, 32]],
                         q2 * 64, dg, [[64, 16], [1, 32]]))
                else:
                    raise AssertionError(l)
        for reg in reg_map.values():
            base = min(reg["_glob"])
            ext = 0
            fixed = []
            for (spb, soff, sap, dpb, dg, dap), g0 in zip(reg["copies"],
                                                          reg["_glob"]):
                rel = g0 - base
                fixed.append((spb, soff, sap, dpb, rel, dap))
                ext = max(ext, rel + sum(st * (ct - 1) for st, ct in dap) + 1)
            assert (base % TILE_W) + ext <= TILE_W, (l, f, base, ext)
            regions.append(dict(p2=reg["p2"], g=reg["g"],
                                dst_start=base % TILE_W, width=ext,
                                copies=fixed))
        del reg_map
    return regions


def layer_slots(l, ly):
    npos = LAYER_NPOS[l]
    nch = max(1, npos // 512)
    return [(ib, m, chk) for (ib, m) in prod_units(l, ly) for chk in range(nch)]


# ------------------------------------------------------------------ mirror
def _ap_cols(off, ap):
    idx = np.zeros((1,), np.int64) + off
    for stride, count in ap:
        idx = (idx[:, None] + (np.arange(count) * stride)[None, :]).reshape(-1)
    return idx


def mirror_core(inputs, c):
    """Pure-numpy mirror of the device plan for core c -> fout [2,128,2048]."""
    u0, v0, ly, blocks = core_geom(c)
    w0 = pack_first(inputs["W0"], u0, v0).astype(np.float32)
    pat = pack_patches(inputs["input_data"]).astype(np.float32)
    wl = {l: pack_weights_layer(inputs[f"W{l}"], l, blocks[l], ly)
          for l in range(1, 6)}
    wf = pack_wf(inputs["Wf"], blocks[5]).astype(np.float32)

    act = {l: [[np.zeros((128, TILE_W), np.float32) for _ in range(NTILES[l])]
               for _ in range(2)] for l in range(1, 6)}
    fact = [None] * 4

    def apply_regions(psum, regions, l_next):
        for reg in regions:
            for (spb, soff, sap, dpb, doff, dap) in reg["copies"]:
                sc = _ap_cols(soff, sap)
                dc = _ap_cols(reg["dst_start"] + doff, dap)
                vals = np.maximum(psum[spb:spb + 64][:, sc], 0.0)
                vals = vals.astype(BF16).astype(np.float32)
                act[l_next][reg["p2"]][reg["g"]][dpb:dpb + 64][:, dc] = vals

    for f in range(8):
        psum = np.zeros((128, FILL_W), np.float32)
        for s in range(4):
            t = 4 * f + s
            psum[:, s * 512:(s + 1) * 512] = w0.T @ pat[:, t * 512:(t + 1) * 512]
        apply_regions(psum, first_fill_descs(f), 1)

    for l in range(1, 6):
        M = LAYER_M[l]
        npos = LAYER_NPOS[l]
        slots = layer_slots(l, ly)
        w_slot = min(npos, 512)
        spf = FILL_W // w_slot
        nfill = len(slots) // spf
        for f in range(nfill):
            psum = np.zeros((128, FILL_W), np.float32)
            for si in range(spf):
                ib, m, chk = slots[f * spf + si]
                colg = ib * npos + chk * 512
                g, loc = colg // TILE_W, colg % TILE_W
                wb = wl[l][ib].astype(np.float32)
                mh = m * 128 if M == 256 else 0
                out = np.zeros((128, w_slot), np.float32)
                for p in range(2):
                    Wh = wb[:, p * M + mh:p * M + mh + 128]
                    Ah = act[l][p][g][:, loc:loc + w_slot]
                    out += Wh.T @ Ah
                psum[:, si * w_slot:(si + 1) * w_slot] = out
            if l == 5:
                fact[f] = np.maximum(psum, 0.0).astype(BF16).astype(np.float32)
            else:
                apply_regions(psum, layer_fill_descs(l, f, ly), l + 1)

    fout = np.zeros((4, 128, FILL_W), np.float32)
    for fi in range(4):
        for j in range(4):
            g = fi * 4 + j
            prod = wf[:, g * 128:(g + 1) * 128].T @ \
                fact[fi][:, j * 512:(j + 1) * 512]
            fout[fi][:, j * 512:(j + 1) * 512] = \
                np.maximum(prod, 0.0).astype(BF16).astype(np.float32)
    return fout


def decode_outputs(fouts):
    """fouts[c] = [4, 128, 2048]: block-diag final-layer product; slot
    s = 16*j + t of fill fi lives at rows 8t+(klx*4+k), cols 512j+32t+b."""
    out = np.zeros((B, C, 2, 64, 64), np.float32)
    for c, fo in fouts.items():
        fo = np.asarray(fo, np.float32)
        _, _, _, blocks = core_geom(c)
        blocks5 = blocks[5]
        for fi in range(4):
            for s in range(64):
                j, t = divmod(s, 16)
                idx = 64 * fi + s
                ib, m = idx // 2, idx % 2
                u, v = blocks5[ib]
                for klx in range(2):
                    U, V = 2 * u + m, 2 * v + klx
                    r0 = 8 * t + klx * 4
                    c0 = 512 * j + 32 * t
                    yf = np.maximum(fo[fi, r0:r0 + 4, c0:c0 + 32], 0.0)
                    out[:, 0, 0, U, V] = yf[0] - yf[2]
                    out[:, 0, 1, U, V] = yf[1] - yf[3]
    return out


def mirror_forward(inputs, cores=range(N_CORES)):
    return decode_outputs({c: mirror_core(inputs, c) for c in cores})


# ------------------------------------------------------------- numpy fallback
def _numpy_reference(inputs):
    x = np.asarray(inputs["input_data"], np.float32)
    b, c_, h, w = x.shape
    xs = np.zeros((b, c_, 4, h, w), np.float32)
    xs[:, :, 0] = x
    p = xs.reshape(b, c_, 4, 32, 2, 32, 2)
    W0 = np.asarray(inputs["W0"], np.float32)
    b0 = np.asarray(inputs["b0"], np.float32)
    y = np.einsum('bcnYhXw,cknhw->bckYX', p, W0) + b0[None, :, :, None, None]
    state = np.maximum(y, 0).reshape(b, c_, 2, 2, NCH, 32, 32)
    for l in range(1, 6):
        Wl = np.asarray(inputs[f"W{l}"], np.float32)
        bl = np.asarray(inputs[f"b{l}"], np.float32)
        G = Wl.shape[1]
        s = state.shape[-1]
        s2 = s // 2
        p = state.reshape(b, c_, G, G, NCH, s2, 2, s2, 2)
        y = np.einsum('bcuvnYpXq,cuvknpq->bcuvkYX', p, Wl) + \
            bl[None, :, :, :, :, None, None]
        y = np.maximum(y, 0).reshape(b, c_, G, G, 2, 2, NCH, s2, s2)
        y = y.transpose(0, 1, 2, 4, 3, 5, 6, 7, 8)
        state = y.reshape(b, c_, 2 * G, 2 * G, NCH, s2, s2)
    st = state.reshape(b, c_, 64, 64, NCH)
    Wf = np.asarray(inputs["Wf"], np.float32)
    bf = np.asarray(inputs["bf"], np.float32)
    yf = np.maximum(np.einsum('bcuvn,cuvkn->bcuvk', st, Wf) + bf[None], 0)
    real = yf[..., 0] - yf[..., 2]
    imag = yf[..., 1] - yf[..., 3]
    return np.stack([real, imag], axis=2)


# ------------------------------------------------------------- bass program
_NC_CACHE = {}


def build_nc(stop_after=None, loop=False):
    import concourse.bass as bass
    import concourse.mybir as mybir
    import concourse.tile as tile
    from concourse import bacc
    import contextlib

    F32 = mybir.dt.float32
    BF = mybir.dt.bfloat16
    Relu = mybir.ActivationFunctionType.Relu

    nc = bacc.Bacc(None, target_bir_lowering=False, debug=True)

    d_pat = nc.dram_tensor("patches", [8, 16384], BF, kind="ExternalInput")
    d_w0 = nc.dram_tensor("w0", [8, 128], BF, kind="ExternalInput")
    d_wl = {l: nc.dram_tensor(f"w{l}", [LAYER_NB[l], 128, 2 * LAYER_M[l]], BF,
                              kind="ExternalInput") for l in range(1, 6)}
    d_wf = nc.dram_tensor("wf", [128, 2048], BF, kind="ExternalInput")
    d_out = nc.dram_tensor("fout", [4, 128, FILL_W], BF, kind="ExternalOutput")
    if loop:
        d_bound = nc.dram_tensor("bound", [1, 1], mybir.dt.int32,
                                 kind="ExternalInput")

    with tile.TileContext(nc) as tc:
        with contextlib.ExitStack() as ctx:
            ps = ctx.enter_context(tc.tile_pool(name="ps", bufs=2, space="PSUM"))
            sb = ctx.enter_context(tc.tile_pool(name="sb", bufs=1))
            wpool = ctx.enter_context(tc.tile_pool(name="wp", bufs=1))

            loop_cm = contextlib.nullcontext()
            if loop:
                bt = sb.tile([1, 1], mybir.dt.int32, tag="bt", bufs=1)
                nc.sync.dma_start(out=bt[:], in_=d_bound[:])
                nval = nc.values_load(bt[0:1, 0:1], min_val=0, max_val=1000000,
                                      skip_runtime_bounds_check=True)
                loop_cm = tc.For_i(0, nval, 1)
            ctx.enter_context(loop_cm)

            w0_sb = sb.tile([8, 128], BF, tag="w0", bufs=1)
            nc.sync.dma_start(out=w0_sb[:], in_=d_w0[:])
            pat_sb = []
            for i in range(4):
                t = sb.tile([8, 4096], BF, tag="pat", bufs=2, name=f"pat{i}")
                nc.sync.dma_start(out=t[:], in_=d_pat[:, i * 4096:(i + 1) * 4096])
                pat_sb.append(t)
            wf_sb = sb.tile([128, 2048], BF, tag="wf", bufs=1)
            nc.sync.dma_start(out=wf_sb[:], in_=d_wf[:])

            act = {l: [[None] * NTILES[l] for _ in range(2)]
                   for l in range(1, 6)}

            def act_tile(l, p, g):
                if act[l][p][g] is None:
                    act[l][p][g] = sb.tile(
                        [128, TILE_W], BF, tag="act", bufs=12,
                        name=f"act{l}_{p}{g}")
                return act[l][p][g]

            scat_flip = [0]

            def relu_copy(dst, src):
                if scat_flip[0] % 4 < ACT_SHARE:
                    nc.scalar.activation(dst, src, Relu)
                else:
                    nc.vector.tensor_scalar_max(dst, src, 0.0)
                scat_flip[0] += 1

            def emit_scatter(psum, regions, l_next):
                if SCATTER_MODE == "fused2":
                    for reg in regions:
                        p2, g, st_loc = reg["p2"], reg["g"], reg["dst_start"]
                        w = min(reg["width"], FILL_W)
                        ah = act_tile(l_next, p2, g)
                        relu_copy(ah[:, st_loc:st_loc + w], psum[:, 0:w])
                    return
                deint = SCATTER_MODE == "bounce2"
                if SCATTER_MODE == "bounce":
                    # relu psum -> bf16 staging in wide ops (frees PSUM
                    # fast); the strided scatter then reads staging
                    stg = sb.tile([128, FILL_W], BF, tag="stg",
                                  bufs=STG_BUFS, name="stg")
                    seg = FILL_W // BOUNCE_SPLIT
                    for h in range(BOUNCE_SPLIT):
                        relu_copy(stg[:, h * seg:(h + 1) * seg],
                                  psum[:, h * seg:(h + 1) * seg])
                    s_tile, s_w = stg, FILL_W
                elif deint:
                    # like bounce, but staging de-interleaves x-parity:
                    # stg col = (c >> 1) + (c & 1)*1024, so the scatter's
                    # inner dims become contiguous (DVE 2x eligible)
                    stg = sb.tile([128, FILL_W], BF, tag="stg", bufs=2,
                                  name="stg")
                    for h in range(2):
                        src = bass.AP(tensor=psum[:].tensor,
                                      offset=psum[:].offset + h * 1024,
                                      ap=[[FILL_W, 128], [2, 512], [1, 2]])
                        dst = bass.AP(tensor=stg[:].tensor,
                                      offset=stg[:].offset + h * 512,
                                      ap=[[FILL_W, 128], [1, 512], [1024, 2]])
                        relu_copy(dst, src)
                    s_tile, s_w = stg, FILL_W
                else:
                    s_tile, s_w = psum, FILL_W
                from_sbuf = SCATTER_MODE in ("bounce", "bounce2")
                for reg in regions:
                    p2, g, st_loc = reg["p2"], reg["g"], reg["dst_start"]
                    ah = act_tile(l_next, p2, g)
                    for (spb, soff, sap, dpb, doff, dap) in reg["copies"]:
                        if deint:
                            soff2 = (soff >> 1) + (soff & 1) * 1024
                            sap2 = [[st // 2, ct] for st, ct in sap]
                        else:
                            soff2, sap2 = soff, [list(x) for x in sap]
                        src = bass.AP(
                            tensor=s_tile[:].tensor,
                            offset=s_tile[:].offset + spb * s_w + soff2,
                            ap=[[s_w, 64]] + sap2)
                        dst_h = bass.AP(
                            tensor=ah[:].tensor,
                            offset=ah[:].offset + dpb * TILE_W + st_loc + doff,
                            ap=[[TILE_W, 64]] + [list(x) for x in dap])
                        r = scat_flip[0] % 4
                        if from_sbuf and r >= 4 - POOL_SHARE:
                            nc.gpsimd.tensor_scalar_max(dst_h, src, 0.0)
                        elif r % 2 == 0:
                            nc.scalar.activation(dst_h, src, Relu)
                        else:
                            nc.vector.tensor_scalar_max(dst_h, src, 0.0)
                        scat_flip[0] += 1

            # first layer
            for f in range(8):
                psum = ps.tile([128, FILL_W], F32, tag="ps", bufs=2, name="psF")
                for s in range(4):
                    t = 4 * f + s
                    rhs = pat_sb[t // 8][:, (t % 8) * 512:(t % 8) * 512 + 512]
                    nc.tensor.matmul(psum[:, s * 512:(s + 1) * 512],
                                     w0_sb[:], rhs, start=True, stop=True)
                emit_scatter(psum, first_fill_descs(f), 1)

            # recursion layers (program identical across cores; ly only
            # affects the data packed on the host)
            fact_tiles = []
            # weight tiles are fetched QGRP blocks per DMA (contiguous in
            # DRAM) to cut DMA queue overhead; bufs are in group units
            wbufs = WBUFS
            w_sbs = {l: {} for l in range(1, 6)}

            def emit_fill(l, f):
                M = LAYER_M[l]
                npos = LAYER_NPOS[l]
                slots = layer_slots(l, 0)
                w_slot = min(npos, 512)
                spf = FILL_W // w_slot
                w_sb = w_sbs[l]
                psum = ps.tile([128, FILL_W], F32, tag="ps", bufs=2,
                               name=f"psl{l}")
                fill_slots = slots[f * spf:(f + 1) * spf]
                Q = QGRP[l]
                for (ib, m, chk) in fill_slots:
                    qi = ib // Q
                    if qi not in w_sb:
                        wt = wpool.tile([128, Q * 2 * M], BF, tag=f"w{l}",
                                        bufs=wbufs[l], name=f"w{l}_q{qi}")
                        src = d_wl[l][qi * Q:(qi + 1) * Q]
                        if Q > 1:
                            src = src.rearrange("i p c -> p i c")
                        dma_eng = nc.scalar if qi % 2 else nc.sync
                        dma_eng.dma_start(out=wt[:], in_=src)
                        w_sb[qi] = wt
                for si, (ib, m, chk) in enumerate(fill_slots):
                    wt = w_sb[ib // Q]
                    wbase = (ib % Q) * 2 * M
                    colg = ib * npos + chk * 512
                    g, loc = colg // TILE_W, colg % TILE_W
                    mh = m * 128 if M == 256 else 0
                    pslice = psum[:, si * w_slot:(si + 1) * w_slot]
                    for p in range(2):
                        lhsT = wt[:, wbase + p * M + mh:
                                  wbase + p * M + mh + 128]
                        rhs = act_tile(l, p, g)[:, loc:loc + w_slot]
                        nc.tensor.matmul(pslice, lhsT, rhs,
                                         start=(p == 0), stop=(p == 1))
                if l == 5:
                    ft = sb.tile([128, FILL_W], BF, tag="fact", bufs=4,
                                 name=f"fact{f}")
                    wseg = FILL_W // RELU_SPLIT
                    for rs in range(RELU_SPLIT):
                        relu_copy(ft[:, rs * wseg:(rs + 1) * wseg],
                                  psum[:, rs * wseg:(rs + 1) * wseg])
                    fact_tiles.append(ft)
                else:
                    emit_scatter(psum, layer_fill_descs(l, f, 0), l + 1)

            def emit_final(fi):
                # final layer: 4 wide matmuls per fill (M=128 = 16 slots x
                # 8 outs, moving 512 = 16 slots x 32 batch); only the
                # diagonal [8t:8t+8, 32t:32t+32] blocks are wanted -- a
                # strided DMA gathers them into d_out (relu'd in SBUF;
                # host applies relu again, idempotent).
                psF = ps.tile([128, FILL_W], F32, tag="ps", bufs=2,
                              name="psfin")
                for j in range(4):
                    g = fi * 4 + j
                    lhsT = wf_sb[:, g * 128:(g + 1) * 128]
                    rhs = fact_tiles[fi][:, j * 512:(j + 1) * 512]
                    nc.tensor.matmul(psF[:, j * 512:(j + 1) * 512],
                                     lhsT, rhs, start=True, stop=True)
                fo = sb.tile([128, FILL_W], BF, tag="fo", bufs=2,
                             name=f"fout{fi}")
                wseg = FILL_W // RELU_SPLIT
                for rs in range(RELU_SPLIT):
                    relu_copy(fo[:, rs * wseg:(rs + 1) * wseg],
                              psF[:, rs * wseg:(rs + 1) * wseg])
                nc.sync.dma_start(out=d_out[fi], in_=fo[:])

            # layers 1-2 layer-major; layers 3-5 + final depth-first per
            # fill chain so l5 weight-ring consumption starts early and
            # the weight DMA stream never stalls
            lvl = {None: 99, "first": 0, "l1": 1, "l2": 2, "l3": 3,
                   "l4": 4, "l5": 5}[stop_after]
            for l in (1, 2):
                if lvl >= l:
                    for f in range(4):
                        emit_fill(l, f)
            for f in range(4):
                for l in (3, 4, 5):
                    if lvl >= l:
                        emit_fill(l, f)
                if stop_after == "l5":
                    nc.sync.dma_start(out=d_out[f], in_=fact_tiles[f][:])
                elif stop_after is None:
                    emit_final(f)
    nc.finalize()
    return nc


# ------------------------------------------------------------------ kernel()
def _pack_in_maps(inputs):
    pat = pack_patches(inputs["input_data"])
    in_maps = []
    for c in range(N_CORES):
        u0, v0, ly, blocks = core_geom(c)
        m = {"patches": pat,
             "w0": pack_first(inputs["W0"], u0, v0),
             "wf": pack_wf(inputs["Wf"], blocks[5])}
        for l in range(1, 6):
            m[f"w{l}"] = pack_weights_layer(inputs[f"W{l}"], l, blocks[l], ly)
        in_maps.append(m)
    return in_maps


def kernel(**inputs):
    exp = {"input_data": (B, C, H, W), "W0": (C, KO, 4, 2, 2), "b0": (C, KO),
           "Wf": (C, 64, 64, 4, NCH), "bf": (C, 64, 64, 4)}
    for l in range(1, 6):
        G = 2 ** l
        exp[f"W{l}"] = (C, G, G, KO, NCH, 2, 2)
        exp[f"b{l}"] = (C, G, G, KO)
    ok = all(tuple(np.shape(inputs.get(k, ()))) == v for k, v in exp.items())
    biases_zero = all(not np.any(np.asarray(inputs[k]))
                      for k in inputs if k.startswith("b"))
    if not ok or not biases_zero:
        return _numpy_reference(inputs)

    from concourse.bass_utils import run_bass_kernel_spmd

    if "nc" not in _NC_CACHE:
        _NC_CACHE["nc"] = build_nc()
    res = run_bass_kernel_spmd(_NC_CACHE["nc"], _pack_in_maps(inputs),
                               core_ids=list(range(N_CORES)))
    return decode_outputs({c: res.results[c]["fout"] for c in range(N_CORES)})

